# revision 1
# baseline (speedup 1.0000x reference)
"""AVAlign kernel for 8 Trainium2 NeuronCores.

Sharding: data-parallel over segments, 4 segments per core (pairs (s, s^1)
stay on-core).

Key algebra vs the reference:
  * cam-weighted pooling commutes with the 1x1 spa_conv, so the 512x512
    conv over [S*F, D, 14, 14] is never done on device:
        tv = (Wv @ Ws) @ pool(feat_v, cam_n) + (bv + Wv bs)
  * temp_conv + max-pool stays: fa = max_hw(Wt @ feat_a), ta = Wa fa + ...
  * ||ta - tv||^2 = ||ta||^2 + ||tv||^2 - 2 ta.tv ; the Gram ta.tv is done
    on device, norms + masking + gathers on host.

Device dtypes: the two big contractions (Wt-conv, cam-pool) run as fp8e4m3
DoubleRow matmuls (k=256 per instruction); Wt is scaled x16 and cam x64 on
host to sit in fp8 range, with the inverse scales folded into Wa and Wv@Ws.
Small matmuls (ta, tv, Gram) are bf16. Max-pool reduces are split between
the DVE (direct reduce_max) and GpSimd (pre-combine halves) engines.
"""

import numpy as np
import ml_dtypes

S, FRAME, CLS, D, DOUT = 32, 8, 32, 512, 128
HWA, HWV = 64, 196           # flattened spatial dims (8*8, 14*14)
HWP = 98                     # HWV folded for DoubleRow: hw = two*98 + p
N_CORES = 8
S_PC = S // N_CORES          # 4 segments per core
BA = S_PC * CLS              # 128 audio rows per core
BV = S_PC * FRAME            # 32 video rows per core
NV = BV * CLS                # 1024 tv rows per core
NB = FRAME * CLS             # 256 tv rows per segment

WT_SCALE = 16.0              # Wt kept x16 in fp8; folded out of Wa
CAM_SCALE = 64.0             # cam_n kept x64 in fp8; folded out of Wv@Ws

_CACHE = {}

# stage-A pair units (16 of (pair-group, oc)).  Two structures, mixed to
# balance DVE vs Act:
#   pure-DVE ("c"): two direct reduce_max from PSUM + tiny bf16 combine
#   Act-drain ("d"): Act copies both psum tiles to bf16, DVE combines at
#                    2x + bf16 chain (TensorTensor allows <=1 PSUM input)
# Put several "c" units in the last pair-group so Act frees up early.
_PURE_DVE = frozenset((3, 8, 12, 14, 15))


def _build_nc():
    from contextlib import ExitStack
    import concourse.bacc as bacc
    import concourse.tile as tile
    import concourse.mybir as mybir

    f32 = mybir.dt.float32
    bf16 = mybir.dt.bfloat16
    f8 = mybir.dt.float8e4
    AX = mybir.AxisListType.X
    DR = mybir.MatmulPerfMode.DoubleRow
    MAXOP = mybir.AluOpType.max
    IDENT = mybir.ActivationFunctionType.Identity

    nc = bacc.Bacc("TRN2", target_bir_lowering=False, debug=False,
                   enable_asserts=False, num_devices=N_CORES)

    # ---- dram tensors -------------------------------------------------
    # feat_a channels packed for DoubleRow: c = dr*256 + i*128 + p
    fa8 = nc.dram_tensor("fa8", [2, 128, 2, BA * HWA], f8, kind="ExternalInput").ap()
    # Wt.T * 16, same channel packing; per dr: [p, (i, oc)]
    wt8 = nc.dram_tensor("wt8", [2, 128, 2 * D], f8, kind="ExternalInput").ap()
    # feat_v packed [hwp, (b, two, d)], hw = two*98 + hwp
    fv8 = nc.dram_tensor("fv8", [HWP, BV * 2 * D], f8, kind="ExternalInput").ap()
    # cam_n * 64 packed [hwp, (b, two, c)]
    cm8 = nc.dram_tensor("cm8", [HWP, BV * 2 * CLS], f8, kind="ExternalInput").ap()
    # (Wv@Ws).T / 64 packed [p, (i, o)] ; Wa.T / 16 packed [p, (i, o)]
    wvs = nc.dram_tensor("wvs", [128, 4 * DOUT], bf16, kind="ExternalInput").ap()
    wab = nc.dram_tensor("wab", [128, 4 * DOUT], bf16, kind="ExternalInput").ap()
    ba2 = nc.dram_tensor("ba2", [DOUT, 1], f32, kind="ExternalInput").ap()
    bv2 = nc.dram_tensor("bv2", [DOUT, 1], f32, kind="ExternalInput").ap()

    f32r = mybir.dt.float32r
    out_ta = nc.dram_tensor("out_ta", [DOUT, BA], f32r, kind="ExternalOutput").ap()
    out_tv = nc.dram_tensor("out_tv", [DOUT, NV], f32r, kind="ExternalOutput").ap()
    # [c, (which, s, f*c)] layout; host transposes
    out_G = nc.dram_tensor("out_G", [CLS, 2 * S_PC * NB], f32, kind="ExternalOutput").ap()

    NG = 8                    # feat_a DMA col groups
    GA = BA * HWA // NG       # 1024 cols per group
    NQ = 4                    # feat_v DMA quarters

    with tile.TileContext(nc) as tc, ExitStack() as ctx:
        wpool = ctx.enter_context(tc.tile_pool(name="weights", bufs=1))
        persist = ctx.enter_context(tc.tile_pool(name="persist", bufs=1))
        apool = ctx.enter_context(tc.tile_pool(name="apool", bufs=1))
        tpool = ctx.enter_context(tc.tile_pool(name="tmp", bufs=5))
        gpool = ctx.enter_context(tc.tile_pool(name="gout", bufs=1))
        ps_a = ctx.enter_context(tc.tile_pool(name="ps_a", bufs=2, space="PSUM"))
        ps_p = ctx.enter_context(tc.tile_pool(name="ps_p", bufs=2, space="PSUM"))
        ps_m = ctx.enter_context(tc.tile_pool(name="ps_m", bufs=2, space="PSUM"))

        # ---- DMA in: weights + first feat_a groups first ----
        wt_sb = []
        for dr in range(2):
            t = wpool.tile([128, 2 * D], f8, tag=f"wt{dr}", name=f"wt{dr}")
            nc.sync.dma_start(t[:], wt8[dr])
            wt_sb.append(t)
        fa_sb = [[None] * 2 for _ in range(NG)]
        def emit_fa_dma(g):
            for dr in range(2):
                t = apool.tile([128, 2 * GA], f8, tag=f"fa{g}_{dr}", name=f"fa{g}_{dr}")
                nc.sync.dma_start(
                    t[:].rearrange("p (i n) -> p i n", i=2),
                    fa8[dr][:, :, g * GA:(g + 1) * GA])
                fa_sb[g][dr] = t

        emit_fa_dma(0)
        emit_fa_dma(4)
        cm_sb = wpool.tile([HWP, BV * 2 * CLS], f8, tag="cm", name="cm_sb")
        fv_sb = persist.tile([HWP, BV * 2 * D], f8, tag="fv", name="fv_sb")
        FVQ = BV * 2 * D // NQ
        def emit_fv_dma(q):
            nc.sync.dma_start(fv_sb[:, q * FVQ:(q + 1) * FVQ],
                              fv8[:, q * FVQ:(q + 1) * FVQ])

        wvs_sb = wpool.tile([128, 4 * DOUT], bf16, tag="wvs", name="wvs_sb")
        wab_sb = wpool.tile([128, 4 * DOUT], bf16, tag="wab", name="wab_sb")
        ba2_sb = wpool.tile([DOUT, 1], f32, tag="ba2", name="ba2_sb")
        bv2_sb = wpool.tile([DOUT, 1], f32, tag="bv2", name="bv2_sb")

        def emit_small_dmas():
            nc.sync.dma_start(cm_sb[:], cm8[:, :])
            nc.sync.dma_start(wvs_sb[:], wvs[:, :])
            nc.sync.dma_start(wab_sb[:], wab[:, :])
            nc.sync.dma_start(ba2_sb[:], ba2[:, :])
            nc.sync.dma_start(bv2_sb[:], bv2[:, :])

        faT = [persist.tile([128, BA], bf16, tag=f"faT{oc}", name=f"faT{oc}")
               for oc in range(4)]
        P_sb = [persist.tile([128, NV], bf16, tag=f"P{i}", name=f"P{i}")
                for i in range(4)]
        tvT = persist.tile([DOUT, NV], f32r, tag="tvT", name="tvT")
        taT = persist.tile([DOUT, BA], f32r, tag="taT", name="taT")
        g_sb = gpool.tile([CLS, 2 * S_PC * NB], f32, tag="gsb", name="g_sb")

        # ---- stage A: fa = max_hw(Wt @ feat_a), fp8 DoubleRow ----
        # feat_a cols are laid out [hw_half, b, hw_lo]: DMA group g covers
        # half g//4, b-block 32*(g%4).  A pair (g, g+4) gives two psum tiles
        # with matching (b, hw_lo) layout, combined element-wise on DVE.
        unit_idx = [0]

        def emit_a_pair(pg):
            fa_A = fa_sb[pg]          # half 0, 32 b's
            fa_B = fa_sb[pg + 4]      # half 1, same b's
            for oc in range(4):
                wt_ap = [wt_sb[dr][:].rearrange("p (i m) -> p i m", i=2)
                         [:, :, oc * 128:(oc + 1) * 128] for dr in range(2)]

                def conv(fa_g, tag):
                    ps = ps_a.tile([128, 1024], f32, tag=tag, name=tag)
                    for q in range(4):
                        for dr in range(2):
                            nc.tensor.matmul(
                                ps[:, q * 256:(q + 1) * 256],
                                wt_ap[dr],
                                fa_g[dr][:].rearrange("p (i n) -> p i n", i=2)
                                    [:, :, q * 256:(q + 1) * 256],
                                start=(dr == 0), stop=(dr == 1),
                                perf_mode=DR)
                    return ps

                u = unit_idx[0]
                unit_idx[0] += 1
                dst = faT[oc][:, pg * 32:(pg + 1) * 32]
                if u in _PURE_DVE:
                    psA = conv(fa_A, "psX")
                    r0 = tpool.tile([128, 32], bf16, tag="r0", name="r0")
                    nc.vector.reduce_max(
                        r0[:], psA[:].rearrange("p (b h) -> p b h", h=32),
                        axis=AX)
                    psB = conv(fa_B, "psX")
                    r1 = tpool.tile([128, 32], bf16, tag="r1", name="r1")
                    nc.vector.reduce_max(
                        r1[:], psB[:].rearrange("p (b h) -> p b h", h=32),
                        axis=AX)
                    nc.vector.tensor_tensor(dst, r0[:], r1[:], op=MAXOP)
                else:
                    psA = conv(fa_A, "psX")
                    Abf = tpool.tile([128, 1024], bf16, tag="Abf", name="Abf")
                    nc.scalar.copy(Abf[:], psA[:])
                    psB = conv(fa_B, "psX")
                    Bbf = tpool.tile([128, 1024], bf16, tag="Bbf", name="Bbf")
                    nc.scalar.copy(Bbf[:], psB[:])
                    M = tpool.tile([128, 1024], bf16, tag="M", name="M")
                    nc.vector.tensor_tensor(M[:], Abf[:], Bbf[:], op=MAXOP)
                    # bf16 chain: [128, 32b, 32hw] -> 16 -> 8 -> reduce
                    m3 = M[:].rearrange("p (b h) -> p b h", h=32)
                    t2 = tpool.tile([128, 512], bf16, tag="t2", name="t2")
                    nc.vector.tensor_tensor(
                        t2[:].rearrange("p (b h) -> p b h", h=16),
                        m3[:, :, 0:16], m3[:, :, 16:32], op=MAXOP)
                    t23 = t2[:].rearrange("p (b h) -> p b h", h=16)
                    t3 = tpool.tile([128, 256], bf16, tag="t3", name="t3")
                    nc.vector.tensor_tensor(
                        t3[:].rearrange("p (b h) -> p b h", h=8),
                        t23[:, :, 0:8], t23[:, :, 8:16], op=MAXOP)
                    nc.vector.reduce_max(
                        dst, t3[:].rearrange("p (b h) -> p b h", h=8), axis=AX)

        # ta column block for one pair-group (emitted late so the PE never
        # stalls waiting on that group's DVE chains mid-stream)
        def emit_ta_block(pg):
            pta = ps_m.tile([DOUT, 32], f32, tag="psM", name="ptab")
            for i in range(4):
                nc.tensor.matmul(pta[:], wab_sb[:, i * DOUT:(i + 1) * DOUT],
                                 faT[i][:, pg * 32:(pg + 1) * 32],
                                 start=(i == 0), stop=(i == 3))
            nc.scalar.activation(taT[:, pg * 32:(pg + 1) * 32], pta[:],
                                 IDENT, bias=ba2_sb[:, 0:1], scale=1.0)

        # ---- stage P: P[i] = pool(feat_v, cam) fp8 DoubleRow, per row ----
        def emit_p_group(i, bg):
            # 16 video rows -> one psum tile [128, 512]
            pp = ps_p.tile([128, 512], f32, tag="psP", name="psP")
            fv3 = fv_sb[:].rearrange("p (b i d) -> p b i d", b=BV, i=2)
            cm3 = cm_sb[:].rearrange("p (b i c) -> p b i c", b=BV, i=2)
            for bl in range(16):
                b = bg * 16 + bl
                nc.tensor.matmul(
                    pp[:, bl * CLS:(bl + 1) * CLS],
                    fv3[:, b][:, :, i * 128:(i + 1) * 128],
                    cm3[:, b],
                    start=True, stop=True, perf_mode=DR)
            nc.scalar.copy(P_sb[i][:, bg * 512:(bg + 1) * 512], pp[:])

        # feat_a pairs get DMA priority; fv/cam stream in behind them since
        # stage P runs late anyway.  A pairs pipeline PE -> Act -> DVE.
        emit_fa_dma(1)
        emit_fa_dma(5)
        emit_a_pair(0)
        emit_fa_dma(2)
        emit_fa_dma(6)
        emit_a_pair(1)
        emit_fa_dma(3)
        emit_fa_dma(7)
        emit_small_dmas()
        emit_fv_dma(0)
        emit_fv_dma(1)
        emit_a_pair(2)
        emit_fv_dma(2)
        emit_fv_dma(3)
        emit_p_group(0, 0)
        emit_p_group(1, 0)
        emit_a_pair(3)
        emit_p_group(2, 0)
        emit_p_group(3, 0)
        emit_p_group(0, 1)
        emit_p_group(1, 1)
        emit_p_group(2, 1)
        emit_p_group(3, 1)
        emit_ta_block(0)
        emit_ta_block(1)
        emit_ta_block(2)

        # ---- tv = (WvWs/64) @ P + bv2 ----
        for hh in range(2):
            pt = ps_m.tile([128, 512], f32, tag="psM", name="psM")
            for i in range(4):
                nc.tensor.matmul(
                    pt[:],
                    wvs_sb[:, i * DOUT:(i + 1) * DOUT],
                    P_sb[i][:, hh * 512:(hh + 1) * 512],
                    start=(i == 0), stop=(i == 3))
            nc.scalar.activation(tvT[:, hh * 512:(hh + 1) * 512], pt[:],
                                 IDENT, bias=bv2_sb[:, 0:1], scale=1.0)
            nc.gpsimd.dma_start(out_tv[:, hh * 512:(hh + 1) * 512],
                                tvT[:, hh * 512:(hh + 1) * 512])

        # ---- ta tail: last pair-group block, then DMA out ----
        emit_ta_block(3)
        nc.gpsimd.dma_start(out_ta[:, :], taT[:])

        # ---- Gram: G[c, (which, s, fb)] = ta[s].T @ tv[s or s^1] ----
        for s in range(S_PC):
            for which, sp in ((0, s), (1, s ^ 1)):
                pg = ps_m.tile([CLS, NB], f32, tag="psM", name="psG")
                nc.tensor.matmul(
                    pg[:],
                    taT[:, s * CLS:(s + 1) * CLS],
                    tvT[:, sp * NB:(sp + 1) * NB],
                    start=True, stop=True)
                eng = nc.vector if (s + which) % 2 == 0 else nc.scalar
                if eng is nc.vector:
                    eng.tensor_copy(
                        g_sb[:, (which * S_PC + s) * NB:
                             (which * S_PC + s + 1) * NB], pg[:])
                else:
                    nc.scalar.copy(
                        g_sb[:, (which * S_PC + s) * NB:
                             (which * S_PC + s + 1) * NB], pg[:])
        nc.sync.dma_start(out_G[:, :], g_sb[:])

    nc.compile()
    return nc


def _get_nc():
    if "nc" not in _CACHE:
        _CACHE["nc"] = _build_nc()
    return _CACHE["nc"]


def _prep_in_maps(inputs):
    f8 = ml_dtypes.float8_e4m3
    bf = ml_dtypes.bfloat16

    # feat_a: [S*C, D, 8, 8] -> per core [2(dr), 128(p), 2(i), BA*HWA]
    # channel c = dr*256 + i*128 + p; cols ordered [hw_half, b, hw_lo] so
    # the two hw-halves form element-aligned psum tiles for the DVE max
    fa = (np.asarray(inputs["feat_a"], np.float32)
          .reshape(N_CORES, BA, D, 2, 32).transpose(0, 2, 3, 1, 4)
          .reshape(N_CORES, D, BA * HWA)
          .reshape(N_CORES, 2, 2, 128, BA * HWA).transpose(0, 1, 3, 2, 4))
    fa8 = np.ascontiguousarray(fa).astype(f8)

    # feat_v: [S*F, D, 14, 14] -> per core [98(hwp), BV, 2(two), D]
    fv = (np.asarray(inputs["feat_v"], np.float32)
          .reshape(N_CORES, BV, D, HWV).transpose(0, 3, 1, 2)
          .reshape(N_CORES, 2, HWP, BV, D).transpose(0, 2, 3, 1, 4)
          .reshape(N_CORES, HWP, BV * 2 * D))
    fv8 = np.ascontiguousarray(fv).astype(f8)

    cam = np.asarray(inputs["cam"], np.float32).reshape(S * FRAME, CLS, HWV)
    cam_n = (cam / (cam.sum(-1, keepdims=True) + 1e-10)) * CAM_SCALE
    cm = (cam_n.reshape(N_CORES, BV, CLS, HWV).transpose(0, 3, 1, 2)
          .reshape(N_CORES, 2, HWP, BV, CLS).transpose(0, 2, 3, 1, 4)
          .reshape(N_CORES, HWP, BV * 2 * CLS))
    cm8 = np.ascontiguousarray(cm).astype(f8)

    Wt = np.asarray(inputs["Wt"], np.float32)
    Ws = np.asarray(inputs["Ws"], np.float32)
    Wa = np.asarray(inputs["Wa"], np.float32)
    Wv = np.asarray(inputs["Wv"], np.float32)
    bt = np.asarray(inputs["bt"], np.float32)
    bs = np.asarray(inputs["bs"], np.float32)
    ba = np.asarray(inputs["ba"], np.float32)
    bv = np.asarray(inputs["bv"], np.float32)

    # Wt.T*16 packed like feat_a channels: wt8[dr][p, i*D + oc]
    wt8 = np.ascontiguousarray(
        (Wt.T * WT_SCALE).reshape(2, 2, 128, D).transpose(0, 2, 1, 3)
        .reshape(2, 128, 2 * D)).astype(f8)

    Wvs = (Wv @ Ws).T / CAM_SCALE          # [D, DOUT]
    wvs = np.ascontiguousarray(
        Wvs.reshape(4, 128, DOUT).transpose(1, 0, 2)
        .reshape(128, 4 * DOUT)).astype(bf)
    WaT = Wa.T / WT_SCALE                  # [D, DOUT]
    wab = np.ascontiguousarray(
        WaT.reshape(4, 128, DOUT).transpose(1, 0, 2)
        .reshape(128, 4 * DOUT)).astype(bf)

    shared = {
        "wt8": wt8, "wvs": wvs, "wab": wab,
        "ba2": np.ascontiguousarray((ba + Wa @ bt).reshape(DOUT, 1)),
        "bv2": np.ascontiguousarray((bv + Wv @ bs).reshape(DOUT, 1)),
    }
    in_maps = []
    for k in range(N_CORES):
        m = dict(shared)
        m["fa8"] = fa8[k]
        m["fv8"] = fv8[k]
        m["cm8"] = cm8[k]
        in_maps.append(m)
    return in_maps


def _assemble(inputs, results):
    pred_a = np.asarray(inputs["pred_a"], np.float32)
    pred_v = np.asarray(inputs["pred_v"], np.float32)
    rf = np.asarray(inputs["rand_frames"])
    rc = np.asarray(inputs["rand_classes"])
    ta = np.concatenate(
        [r["out_ta"].T.reshape(S_PC, CLS, DOUT) for r in results])
    tv = np.concatenate(
        [r["out_tv"].T.reshape(S_PC, FRAME, CLS, DOUT) for r in results])
    G = np.concatenate(
        [r["out_G"].reshape(CLS, 2, S_PC, NB).transpose(1, 2, 0, 3)
         for r in results], axis=1)                          # [2, S, C, 256]
    Gself, Gcross = G[0], G[1]

    tan = np.einsum('sco,sco->sc', ta, ta)                   # [S, C]
    tvn = np.einsum('sfco,sfco->sfc', tv, tv)                # [S, F, C]

    pv = 1.0 / (1.0 + np.exp(-pred_v.reshape(S, FRAME, CLS)))
    active_a = pred_a > 0.3
    active_v = pv > 0.3
    c_idx = np.arange(CLS)
    f_idx = np.arange(FRAME)

    G4 = Gself.reshape(S, CLS, FRAME, CLS)
    Gco = G4[:, c_idx[:, None], f_idx[None, :], c_idx[:, None]]  # [S, C, F]
    mask_co = active_a[:, :, None] & active_v.transpose(0, 2, 1)
    loss_co = (tan[:, :, None] + tvn.transpose(0, 2, 1) - 2.0 * Gco) / DOUT
    loss_co = loss_co * mask_co

    j = rf * CLS + rc                                        # [S, C, F]
    Gdi = np.take_along_axis(Gcross, j.reshape(S, CLS, FRAME), axis=2)
    tvn_p = tvn.reshape(S, FRAME * CLS)[np.arange(S) ^ 1]    # partner norms
    tvn_di = np.take_along_axis(tvn_p[:, None, :].repeat(CLS, 1),
                                j.reshape(S, CLS, FRAME), axis=2)
    num = (pred_a * FRAME).astype(np.int32)
    mask_di = active_a[:, :, None] & (f_idx[None, None, :] < num[:, :, None])
    loss_di = (tan[:, :, None] + tvn_di - 2.0 * Gdi) / DOUT
    loss_di = loss_di * mask_di

    return np.stack([loss_co, loss_di]).astype(np.float32)   # [2, S, C, F]


def _run(inputs, trace=False):
    from concourse.bass_utils import run_bass_kernel_spmd
    nc = _get_nc()
    in_maps = _prep_in_maps(inputs)
    try:
        br = run_bass_kernel_spmd(nc, in_maps, list(range(N_CORES)), trace=trace)
    except ModuleNotFoundError:
        br = run_bass_kernel_spmd(nc, in_maps, list(range(N_CORES)), trace=False)
    return _assemble(inputs, br.results), br


def kernel(**inputs):
    out, _ = _run(inputs)
    return out



# revision 4
# speedup vs baseline: 1.1712x; 1.1712x over previous
"""AVAlign kernel for 8 Trainium2 NeuronCores.

Sharding: data-parallel over segments, 4 segments per core (pairs (s, s^1)
stay on-core).  The host computes masks/gathers/norms AND the final Gram
(ta.tv dot products, ~270 MFLOP of BLAS); the device does the two big
contractions:

  * stage A: fa = max_hw(Wt @ feat_a) as fp8 DoubleRow matmuls; only the
    audio rows with pred_a > 0.3 are computed (the rest are masked to zero
    in the output anyway) -- the active set is read from the inputs at
    first call and the module is compiled for that capacity.
  * stage P: cam-weighted pooling of feat_v (fp8 DoubleRow, hw contracted),
    then tv = (Wv@Ws) @ P + bias in bf16.

Stage-A PSUM drain (the expensive part) is split across engines:
  mode-D units: DVE reduce_max straight from PSUM  [128, 32b, 64hw]->[128,32]
  mode-A units: Act copies PSUM -> SBUF bf16, DVE runs a tensor_tensor
                max tree at its 2x bf16 rate.
Ratios are chosen so DVE / Act / DMA all finish together.

Outputs per core: ta [DOUT, NB] f32, tv [DOUT, 1024] f32.  Host unpacks the
active rows, computes ||ta||^2, ||tv||^2, the self/cross Grams, and the
masked dense loss layout exactly as the reference does.
"""

import numpy as np
import ml_dtypes

S, FRAME, CLS, D, DOUT = 32, 8, 32, 512, 128
HWA, HWV, HWP = 64, 196, 98
N_CORES = 8
S_PC = S // N_CORES          # 4 segments per core
BV = S_PC * FRAME            # 32 video rows per core
NV = BV * CLS                # 1024 tv rows per core

WT_SCALE = 16.0              # Wt kept x16 in fp8; folded out of Wa
CAM_SCALE = 64.0             # cam_n kept x64 in fp8; folded out of Wv@Ws

_CACHE = {}


def _unit_sizes(nb):
    """Split nb audio rows into units of 32 (plus an 8-granular remainder)."""
    sizes = [32] * (nb // 32)
    if nb % 32:
        sizes.append(nb % 32)
    return sizes


def _build_nc(nb):
    from contextlib import ExitStack
    import concourse.bacc as bacc
    import concourse.tile as tile
    import concourse.mybir as mybir

    f32 = mybir.dt.float32
    bf16 = mybir.dt.bfloat16
    f8 = mybir.dt.float8e4
    AX = mybir.AxisListType.X
    DR = mybir.MatmulPerfMode.DoubleRow
    MAXOP = mybir.AluOpType.max
    IDENT = mybir.ActivationFunctionType.Identity

    units = _unit_sizes(nb)
    NU = len(units)
    # mode assignment for full (32b) units: per (u, oc), True = Act-copy path.
    # Balance: DVE direct 2258ns/unit vs (Act 1892 + DVE tree 1410).
    n_full = sum(1 for s in units if s == 32)
    act_path = {}
    k = 0
    for u in range(NU):
        for oc in range(4):
            if units[u] == 32:
                # 2 of every 3 full units on the Act path
                act_path[(u, oc)] = (k % 3) != 0
                k += 1
            else:
                act_path[(u, oc)] = False   # small remainder units: direct DVE

    nc = bacc.Bacc("TRN2", target_bir_lowering=False, debug=False,
                   enable_asserts=False, num_devices=N_CORES)

    # ---- dram tensors -------------------------------------------------
    # feat_a channels packed c = dr*256 + i*128 + p; cols b-major, hw contig
    fa8 = nc.dram_tensor("fa8", [128, 2, 2, nb * HWA], f8,
                         kind="ExternalInput").ap()
    # Wt.T * 16, same channel packing: [p, dr, i, oc]
    wt8 = nc.dram_tensor("wt8", [128, 2, 2, D], f8, kind="ExternalInput").ap()
    # feat_v packed [hwp, b, two, d], hw = two*98 + hwp
    fv8 = nc.dram_tensor("fv8", [HWP, BV, 2, D], f8, kind="ExternalInput").ap()
    # cam_n * 64 packed [hwp, b, two, c]
    cm8 = nc.dram_tensor("cm8", [HWP, BV, 2, CLS], f8,
                         kind="ExternalInput").ap()
    # (Wv@Ws).T / 64 packed [p, i, o] ; Wa.T / 16 packed [p, i, o]
    wvs = nc.dram_tensor("wvs", [128, 4, DOUT], bf16, kind="ExternalInput").ap()
    wab = nc.dram_tensor("wab", [128, 4, DOUT], bf16, kind="ExternalInput").ap()
    # col 0 = ba + Wa@bt, col 1 = bv + Wv@bs
    b2 = nc.dram_tensor("b2", [DOUT, 2], f32, kind="ExternalInput").ap()

    out_ta = nc.dram_tensor("out_ta", [DOUT, nb], f32, kind="ExternalOutput").ap()
    out_tv = nc.dram_tensor("out_tv", [DOUT, NV], f32, kind="ExternalOutput").ap()

    with tile.TileContext(nc) as tc, ExitStack() as ctx:
        wpool = ctx.enter_context(tc.tile_pool(name="weights", bufs=1))
        persist = ctx.enter_context(tc.tile_pool(name="persist", bufs=1))
        cpool = ctx.enter_context(tc.tile_pool(name="cp", bufs=2))
        tpool = ctx.enter_context(tc.tile_pool(name="tree", bufs=2))
        # two rotating 4-bank PSUM slots hold every matmul target
        psU = ctx.enter_context(tc.tile_pool(name="psU", bufs=2, space="PSUM"))

        wt_sb = wpool.tile([128, 2, 2, D], f8, tag="wt", name="wt_sb")
        fa_sb = persist.tile([128, 2, 2, nb * HWA], f8, tag="fa", name="fa_sb")
        fv_sb = persist.tile([HWP, BV, 2, D], f8, tag="fv", name="fv_sb")
        cm_sb = wpool.tile([HWP, BV, 2, CLS], f8, tag="cm", name="cm_sb")
        wvs_sb = wpool.tile([128, 4, DOUT], bf16, tag="wvs", name="wvs_sb")
        wab_sb = wpool.tile([128, 4, DOUT], bf16, tag="wab", name="wab_sb")
        b2_sb = wpool.tile([DOUT, 2], f32, tag="b2", name="b2_sb")

        faT = [persist.tile([128, nb], bf16, tag=f"faT{i}", name=f"faT{i}")
               for i in range(4)]
        P_sb = [persist.tile([128, NV], bf16, tag=f"P{i}", name=f"P{i}")
                for i in range(4)]
        tvT = persist.tile([DOUT, NV], f32, tag="tvT", name="tvT")
        taT = persist.tile([DOUT, nb], f32, tag="taT", name="taT")

        ustart = [0]
        for sz in units:
            ustart.append(ustart[-1] + sz)

        # ---- DMA emissions (SP sequencer; order sets device priority) ----
        def dma_fa(u):
            c0, c1 = ustart[u] * HWA, ustart[u + 1] * HWA
            for dr in range(2):
                nc.sync.dma_start(fa_sb[:, dr, :, c0:c1], fa8[:, dr, :, c0:c1])

        def dma_fv(q):
            nc.sync.dma_start(fv_sb[:, q * 8:(q + 1) * 8], fv8[:, q * 8:(q + 1) * 8])

        # ---- stage A unit: conv 512->128oc over [ub b x 64 hw] ----------
        def emit_a_unit(u, oc):
            ub = units[u]
            cols = ub * HWA
            c0 = ustart[u] * HWA
            ps = psU.tile([128, 2048], f32, tag="u", name=f"A{u}_{oc}")
            wt_ap = [wt_sb[:, dr, :, oc * 128:(oc + 1) * 128] for dr in range(2)]
            for q in range(cols // 256):
                for dr in range(2):
                    nc.tensor.matmul(
                        ps[:, q * 256:(q + 1) * 256],
                        wt_ap[dr],
                        fa_sb[:, dr, :, c0 + q * 256:c0 + (q + 1) * 256],
                        start=(dr == 0), stop=(dr == 1), perf_mode=DR)
            dst = faT[oc][:, ustart[u]:ustart[u + 1]]
            if not act_path[(u, oc)]:
                nc.vector.reduce_max(
                    dst, ps[:, 0:cols].rearrange("p (b h) -> p b h", h=HWA),
                    axis=AX)
            else:
                cp = cpool.tile([128, 2048], bf16, tag="cp", name="cp")
                nc.scalar.copy(cp[:, 0:cols], ps[:, 0:cols])
                cur = cp[:, 0:cols].rearrange("p (b h) -> p b h", h=HWA)
                width = HWA
                lvl = 0
                while width > 2:
                    half = width // 2
                    t = tpool.tile([128, ub * half], bf16, tag=f"t{lvl}",
                                   name=f"t{lvl}")
                    t3 = t[:].rearrange("p (b h) -> p b h", h=half)
                    nc.vector.tensor_tensor(t3, cur[:, :, 0:half],
                                            cur[:, :, half:width], op=MAXOP)
                    cur = t3
                    width = half
                    lvl += 1
                nc.vector.tensor_tensor(dst, cur[:, :, 0], cur[:, :, 1],
                                        op=MAXOP)

        # ---- stage P: P[i][:, (b,c)] = pool(feat_v, cam) ---------------
        def emit_p_tile(i, bh):
            pp = psU.tile([128, 2048], f32, tag="u", name=f"P{i}_{bh}")
            for bl in range(16):
                b = bh * 16 + bl
                nc.tensor.matmul(
                    pp[:, bl * CLS:(bl + 1) * CLS],
                    fv_sb[:, b, :, i * 128:(i + 1) * 128],
                    cm_sb[:, b],
                    start=True, stop=True, perf_mode=DR)
            nc.scalar.copy(P_sb[i][:, bh * 512:(bh + 1) * 512], pp[:, 0:512])

        # ---- tv = (WvWs/64) @ P + bv2 ----------------------------------
        def emit_tv():
            pt = psU.tile([128, 2048], f32, tag="u", name="tv")
            for hh in range(2):
                for i in range(4):
                    nc.tensor.matmul(pt[:, hh * 512:(hh + 1) * 512],
                                     wvs_sb[:, i],
                                     P_sb[i][:, hh * 512:(hh + 1) * 512],
                                     start=(i == 0), stop=(i == 3))
            nc.scalar.activation(tvT[:], pt[:, 0:NV], IDENT,
                                 bias=b2_sb[:, 1:2], scale=1.0)

        # ---- ta = (Wa/16) @ fa + ba2 -----------------------------------
        def emit_ta():
            pt = psU.tile([128, 2048], f32, tag="u", name="ta")
            for i in range(4):
                nc.tensor.matmul(pt[:, 0:nb], wab_sb[:, i], faT[i][:],
                                 start=(i == 0), stop=(i == 3))
            nc.scalar.activation(taT[:], pt[:, 0:nb], IDENT,
                                 bias=b2_sb[:, 0:1], scale=1.0)

        # ---- schedule ---------------------------------------------------
        nc.sync.dma_start(wt_sb[:], wt8[:, :, :, :])
        dma_fa(0)
        nc.sync.dma_start(cm_sb[:], cm8[:, :, :, :])
        dma_fa(1)
        nc.sync.dma_start(wvs_sb[:], wvs[:, :, :])
        nc.sync.dma_start(wab_sb[:], wab[:, :, :])
        nc.sync.dma_start(b2_sb[:], b2[:, :])
        dma_fv(0)
        if NU > 2:
            dma_fa(2)
        dma_fv(1)
        if NU > 3:
            dma_fa(3)
        dma_fv(2)
        for u in range(4, NU):
            dma_fa(u)
        dma_fv(3)

        # A-units for b-unit 0/1 first; interleave P after fv halves land.
        for oc in range(4):
            emit_a_unit(0, oc)
        for oc in range(4):
            emit_a_unit(1, oc)
        for i in range(4):
            emit_p_tile(i, 0)
        if NU > 2:
            for oc in range(4):
                emit_a_unit(2, oc)
        for i in range(4):
            emit_p_tile(i, 1)
        emit_tv()
        nc.sync.dma_start(out_tv[:, :], tvT[:])
        for u in range(3, NU):
            for oc in range(4):
                emit_a_unit(u, oc)
        emit_ta()
        nc.sync.dma_start(out_ta[:, :], taT[:])

    nc.compile()
    return nc


def _get_nc(nb):
    key = ("nc", nb)
    if key not in _CACHE:
        _CACHE[key] = _build_nc(nb)
    return _CACHE[key]


def _active_layout(pred_a):
    """Per-core active (seg, class) lists and the common capacity NB."""
    active = np.asarray(pred_a, np.float32) > 0.3
    rows = []
    for k in range(N_CORES):
        lst = []
        for sp in range(S_PC):
            s = k * S_PC + sp
            for c in range(CLS):
                if active[s, c]:
                    lst.append((s, c))
        rows.append(lst)
    nmax = max(max(len(r) for r in rows), 8)
    nb = ((nmax + 7) // 8) * 8
    return rows, nb


def _prep_in_maps(inputs, rows, nb):
    f8 = ml_dtypes.float8_e4m3
    bf = ml_dtypes.bfloat16

    feat_a = np.asarray(inputs["feat_a"], np.float32).reshape(S * CLS, D, HWA)
    fa_packed = np.zeros((N_CORES, nb, D, HWA), np.float32)
    for k in range(N_CORES):
        idx = [s * CLS + c for (s, c) in rows[k]]
        fa_packed[k, :len(idx)] = feat_a[idx]
    # [k, b, (dr, i, p), hw] -> [k, p, dr, i, b*hw]
    fa8 = np.ascontiguousarray(
        fa_packed.reshape(N_CORES, nb, 2, 2, 128, HWA)
        .transpose(0, 4, 2, 3, 1, 5)
        .reshape(N_CORES, 128, 2, 2, nb * HWA)).astype(f8)

    fv = (np.asarray(inputs["feat_v"], np.float32)
          .reshape(N_CORES, BV, D, HWV).transpose(0, 3, 1, 2)
          .reshape(N_CORES, 2, HWP, BV, D).transpose(0, 2, 3, 1, 4))
    fv8 = np.ascontiguousarray(fv).astype(f8)    # [k, 98, 32, 2, 512]

    cam = np.asarray(inputs["cam"], np.float32).reshape(S * FRAME, CLS, HWV)
    cam_n = (cam / (cam.sum(-1, keepdims=True) + 1e-10)) * CAM_SCALE
    cm = (cam_n.reshape(N_CORES, BV, CLS, HWV).transpose(0, 3, 1, 2)
          .reshape(N_CORES, 2, HWP, BV, CLS).transpose(0, 2, 3, 1, 4))
    cm8 = np.ascontiguousarray(cm).astype(f8)    # [k, 98, 32, 2, 32]

    Wt = np.asarray(inputs["Wt"], np.float32)
    Ws = np.asarray(inputs["Ws"], np.float32)
    Wa = np.asarray(inputs["Wa"], np.float32)
    Wv = np.asarray(inputs["Wv"], np.float32)
    bt = np.asarray(inputs["bt"], np.float32)
    bs = np.asarray(inputs["bs"], np.float32)
    ba = np.asarray(inputs["ba"], np.float32)
    bv = np.asarray(inputs["bv"], np.float32)

    wt8 = np.ascontiguousarray(
        (Wt.T * WT_SCALE).reshape(2, 2, 128, D).transpose(2, 0, 1, 3)).astype(f8)
    wvs = np.ascontiguousarray(
        ((Wv @ Ws).T / CAM_SCALE).reshape(4, 128, DOUT).transpose(1, 0, 2)
    ).astype(bf)
    wab = np.ascontiguousarray(
        (Wa.T / WT_SCALE).reshape(4, 128, DOUT).transpose(1, 0, 2)).astype(bf)
    b2 = np.ascontiguousarray(
        np.stack([ba + Wa @ bt, bv + Wv @ bs], axis=1))

    shared = {"wt8": wt8, "wvs": wvs, "wab": wab, "b2": b2}
    in_maps = []
    for k in range(N_CORES):
        m = dict(shared)
        m["fa8"] = fa8[k]
        m["fv8"] = fv8[k]
        m["cm8"] = cm8[k]
        in_maps.append(m)
    return in_maps


def _assemble(inputs, results, rows, nb):
    pred_a = np.asarray(inputs["pred_a"], np.float32)
    pred_v = np.asarray(inputs["pred_v"], np.float32)
    rf = np.asarray(inputs["rand_frames"])
    rc = np.asarray(inputs["rand_classes"])

    ta_full = np.zeros((S, CLS, DOUT), np.float32)
    for k in range(N_CORES):
        ta_k = results[k]["out_ta"].T          # [nb, DOUT]
        for j, (s, c) in enumerate(rows[k]):
            ta_full[s, c] = ta_k[j]
    tv4 = np.concatenate(
        [r["out_tv"].T.reshape(S_PC, FRAME, CLS, DOUT) for r in results])

    tan = np.einsum('sco,sco->sc', ta_full, ta_full)          # [S, C]
    tvn = np.einsum('sfco,sfco->sfc', tv4, tv4)               # [S, F, C]

    tv_flat = tv4.reshape(S, FRAME * CLS, DOUT)
    Gself = np.matmul(ta_full, tv_flat.transpose(0, 2, 1))    # [S, C, 256]
    rank = np.arange(S) ^ 1
    Gcross = np.matmul(ta_full, tv_flat[rank].transpose(0, 2, 1))

    pv = 1.0 / (1.0 + np.exp(-pred_v.reshape(S, FRAME, CLS)))
    active_a = pred_a > 0.3
    active_v = pv > 0.3
    f_idx = np.arange(FRAME)
    c_idx = np.arange(CLS)

    G4 = Gself.reshape(S, CLS, FRAME, CLS)
    Gco = G4[:, c_idx[:, None], f_idx[None, :], c_idx[:, None]]   # [S, C, F]
    mask_co = active_a[:, :, None] & active_v.transpose(0, 2, 1)
    loss_co = (tan[:, :, None] + tvn.transpose(0, 2, 1) - 2.0 * Gco) / DOUT
    loss_co = loss_co * mask_co

    j = rf * CLS + rc                                             # [S, C, F]
    Gdi = np.take_along_axis(Gcross, j.reshape(S, CLS, FRAME), axis=2)
    tvn_p = tvn.reshape(S, FRAME * CLS)[rank]
    tvn_di = np.take_along_axis(tvn_p[:, None, :].repeat(CLS, 1),
                                j.reshape(S, CLS, FRAME), axis=2)
    num = (pred_a * FRAME).astype(np.int32)
    mask_di = active_a[:, :, None] & (f_idx[None, None, :] < num[:, :, None])
    loss_di = (tan[:, :, None] + tvn_di - 2.0 * Gdi) / DOUT
    loss_di = loss_di * mask_di

    return np.stack([loss_co, loss_di]).astype(np.float32)    # [2, S, C, F]


def _run(inputs, trace=False):
    from concourse.bass_utils import run_bass_kernel_spmd
    rows, nb = _active_layout(inputs["pred_a"])
    nc = _get_nc(nb)
    _CACHE["last_nc"] = nc
    in_maps = _prep_in_maps(inputs, rows, nb)
    try:
        br = run_bass_kernel_spmd(nc, in_maps, list(range(N_CORES)), trace=trace)
    except ModuleNotFoundError:
        br = run_bass_kernel_spmd(nc, in_maps, list(range(N_CORES)), trace=False)
    return _assemble(inputs, br.results, rows, nb), br


def kernel(**inputs):
    out, _ = _run(inputs)
    return out
